# revision 1
# baseline (speedup 1.0000x reference)
"""DGCNN (2x dynamic-kNN EdgeConv + global mean pool + MLP) fully on
Trainium2, 8 NeuronCores, data-parallel over the 64 graphs (8 per core).

On device per core: score matmuls -> DVE top-10 (max8/match_replace/
max_index) -> index re-wrap via PE transpose + DRAM roundtrip -> gpsimd
ap_gather -> pair MLPs with PSUM k-accumulation -> pooled classifier.
The PJRT executable, weight-derived device arrays, and output buffers are
cached across calls; per call only the node features (F1) are transferred.
"""
import sys

sys.path.insert(0, "/opt/trn_rl_repo")
sys.path.insert(0, "/opt/trn_rl_repo/concourse")

import hashlib
import numpy as np
from contextlib import ExitStack

import concourse.mybir as mybir
from concourse import bacc, bass
from concourse.tile import TileContext

NPG = 1024
K = 10
GPC = 8
SLOPE = 0.01
N_CORES = 8

dt = mybir.dt
F32 = dt.float32
F16 = dt.float16
I16 = dt.int16
U16 = dt.uint16


def build(num_devices=N_CORES):
    nc = bacc.Bacc("TRN2", target_bir_lowering=False, debug=False,
                   num_devices=num_devices)
    AF = mybir.ActivationFunctionType
    LRELU, IDENT = AF.Lrelu, AF.Identity

    def din(name, shape, dtype=F32):
        return nc.dram_tensor(name, shape, dtype, kind="ExternalInput").ap()

    F1 = din("F1", [5, GPC * NPG])          # rows 0-3 xxT, row 4 sq
    w1d = din("w1d", [4, 64])               # w1a[:4] - w1a[4:]
    w1bot = din("w1bot", [4, 64])           # w1a[4:]
    w1bw = din("w1bw", [64, 64])
    w1cw = din("w1cw", [64, 64])
    b1a = din("b1a", [64, 1])
    b1b = din("b1b", [64, 1])
    b1c = din("b1c", [64, 1])
    w2d = din("w2d", [64, 128])
    w2b = din("w2b", [64, 128])
    b2 = din("b2", [128, 1])
    wlA = din("wlA", [64, 1024])
    wlB = din("wlB", [128, 1024])
    blr = din("blr", [128, 8])
    wm1r = din("wm1r", [128, 4096])
    bm1r = din("bm1r", [128, 4])
    wm2r = din("wm2r", [128, 1024])
    bm2r = din("bm2r", [128, 2])
    wm3r = din("wm3r", [128, 6])
    bm3r = din("bm3r", [3, 1])
    ident16d = din("ident16", [128, 128], F16)
    out = nc.dram_tensor("outT", [3, GPC], F32, kind="ExternalOutput").ap()

    idx1s = nc.dram_tensor("idx1s", [GPC * 640 * 16], I16, kind="Internal")
    idx2s = nc.dram_tensor("idx2s", [GPC * 640 * 16], I16, kind="Internal")

    with TileContext(nc) as tc:
        ctx = ExitStack()
        cst = ctx.enter_context(tc.tile_pool(name="cst", bufs=1))
        sb = ctx.enter_context(tc.tile_pool(name="sb", bufs=2))
        wk = ctx.enter_context(tc.tile_pool(name="wk", bufs=1))
        big = ctx.enter_context(tc.tile_pool(name="big", bufs=1))
        psc = ctx.enter_context(tc.tile_pool(name="psc", bufs=1, space="PSUM"))
        ppr = ctx.enter_context(tc.tile_pool(name="ppr", bufs=2, space="PSUM"))
        pac = ctx.enter_context(tc.tile_pool(name="pac", bufs=1, space="PSUM"))

        def load_const(ap_in, shape, dtype=F32):
            t = cst.tile(shape, dtype, tag=ap_in.name)
            nc.sync.dma_start(out=t, in_=ap_in)
            return t

        w1d_s = load_const(w1d, [4, 64])
        w1bot_s = load_const(w1bot, [4, 64])
        w1bw_s = load_const(w1bw, [64, 64])
        w1cw_s = load_const(w1cw, [64, 64])
        b1a_s = load_const(b1a, [64, 1])
        b1b_s = load_const(b1b, [64, 1])
        b1c_s = load_const(b1c, [64, 1])
        w2d_s = load_const(w2d, [64, 128])
        w2b_s = load_const(w2b, [64, 128])
        b2_s = load_const(b2, [128, 1])
        wlA_s = load_const(wlA, [64, 1024])
        wlB_s = load_const(wlB, [128, 1024])
        blr_s = load_const(blr, [128, 8])
        wm1_s = load_const(wm1r, [128, 4096])
        bm1_s = load_const(bm1r, [128, 4])
        wm2_s = load_const(wm2r, [128, 1024])
        bm2_s = load_const(bm2r, [128, 2])
        wm3_s = load_const(wm3r, [128, 6])
        bm3_s = load_const(bm3r, [3, 1])
        ident16 = load_const(ident16d, [128, 128], F16)

        F1s = cst.tile([5, GPC * NPG], F32, tag="F1s")
        nc.sync.dma_start(out=F1s, in_=F1)

        identf = cst.tile([128, 128], F32, tag="identf")
        nc.scalar.copy(identf, ident16)

        ones64 = cst.tile([64, 1], F32, tag="ones64")
        nc.vector.memset(ones64, 1.0)
        neghalf = cst.tile([1, 128], F32, tag="neghalf")
        nc.vector.memset(neghalf, -0.5)

        pooled1 = cst.tile([64, GPC], F32, tag="pooled1")
        pooled2 = cst.tile([128, GPC], F32, tag="pooled2")

        def topk_tile(sc, asm, t):
            """sc: [128, NPG] scores (PSUM). Writes top-16 idx into asm cols."""
            v16 = sb.tile([128, 16], F32, tag="v16")
            scratch = wk.tile([128, NPG], F32, tag="scratch")
            nc.vector.max(out=v16[:, 0:8], in_=sc)
            outa = asm[:, 0:64].rearrange("p (k t) -> p k t", t=8)[:, :, t]
            nc.vector.max_index(outa, v16[:, 0:8], sc)
            nc.vector.match_replace(out=scratch, in_to_replace=v16[:, 0:8],
                                    in_values=sc, imm_value=-1e30)
            nc.vector.max(out=v16[:, 8:16], in_=scratch)
            outb = asm[:, 64:128].rearrange("p (k t) -> p k t", t=8)[:, :, t]
            nc.vector.max_index(outb, v16[:, 8:16], scratch)

        def idx_to_dram(asm, stage, g):
            """asm [128, 128] u16 (cols c=k*8+t) -> DRAM wrapped layout."""
            asm_h = sb.tile([128, 128], F16, tag="asm_h")
            nc.vector.tensor_copy(asm_h, asm)
            pt = psc.tile([128, 128], F16, tag="sc")
            nc.tensor.transpose(pt, asm_h, ident16)
            asm_i = sb.tile([128, 128], I16, tag="asm_i")
            nc.scalar.copy(asm_i, pt)
            dst = bass.AP(stage, g * 10240, [[128, 80], [16, 8], [1, 16]])
            nc.sync.dma_start(out=dst, in_=asm_i[0:80, :])

        def mm2(pm, lhsT, rhs, start=True, stop=True):
            for h in range(2):
                nc.tensor.matmul(pm[:, 512 * h:512 * (h + 1)], lhsT,
                                 rhs[:, 512 * h:512 * (h + 1)],
                                 start=start, stop=stop)

        for g in range(GPC):
            gsl = slice(NPG * g, NPG * (g + 1))

            # ---- conv1 scores + topk ----
            ahat_g = wk.tile([5, NPG], F32, tag="ahat")
            nc.vector.memset(ahat_g, -1.0)
            nc.scalar.mul(ahat_g[0:4, :], F1s[0:4, gsl], 2.0)
            asm = sb.tile([128, 128], U16, tag="asm")
            for t in range(8):
                sc = psc.tile([128, NPG], F32, tag="sc")
                mm2(sc, ahat_g[:, 128 * t:128 * (t + 1)], F1s[:, gsl])
                topk_tile(sc, asm, t)
            idx_to_dram(asm, idx1s, g)

            # ---- conv1 u1/v1 ----
            u1p = ppr.tile([64, NPG], F32, tag="pair")
            mm2(u1p, w1d_s, F1s[0:4, gsl])
            u1s = wk.tile([64, NPG], F32, tag="u1s")
            nc.scalar.activation(u1s, u1p, IDENT, bias=b1a_s)
            v1p = ppr.tile([64, NPG], F32, tag="pair")
            mm2(v1p, w1bot_s, F1s[0:4, gsl])
            v1s = wk.tile([64, NPG], F32, tag="v1s")
            nc.scalar.copy(v1s, v1p)

            # ---- conv1 gather ----
            idxw1 = sb.tile([64, 640], I16, tag="idxw1")
            for u in range(4):
                srcap = bass.AP(idx1s, g * 10240, [[1, 16], [16, 640]])
                nc.sync.dma_start(out=idxw1[16 * u:16 * (u + 1), :], in_=srcap)
            vg1 = big.tile([64, K * NPG], F32, tag="gath")
            nc.gpsimd.ap_gather(vg1, v1s, idxw1, channels=64, num_elems=NPG,
                                d=1, num_idxs=K * NPG)

            # ---- conv1 pair MLP ----
            x1acc = pac.tile([64, NPG], F32, tag="acc")
            for k in range(K):
                z1 = wk.tile([64, NPG], F32, tag="z1")
                nc.vector.tensor_add(z1, u1s, vg1[:, NPG * k:NPG * (k + 1)])
                h1 = wk.tile([64, NPG], F32, tag="h1")
                nc.scalar.activation(h1, z1, LRELU, alpha=SLOPE)
                l2 = ppr.tile([64, NPG], F32, tag="pair")
                mm2(l2, w1bw_s, h1)
                h2 = wk.tile([64, NPG], F32, tag="h2")
                nc.scalar.activation(h2, l2, LRELU, bias=b1b_s, alpha=SLOPE)
                l3 = ppr.tile([64, NPG], F32, tag="pair")
                mm2(l3, w1cw_s, h2)
                h3 = wk.tile([64, NPG], F32, tag="h3")
                nc.scalar.activation(h3, l3, LRELU, bias=b1c_s, alpha=SLOPE)
                mm2(x1acc, identf[0:64, 0:64], h3,
                    start=(k == 0), stop=(k == K - 1))
            x1g = wk.tile([64, NPG], F32, tag="x1g")
            nc.scalar.activation(x1g, x1acc, IDENT,
                                 accum_out=pooled1[:, g:g + 1])

            # ---- conv2 prep ----
            x1sq = wk.tile([64, NPG], F32, tag="h1")
            nc.scalar.square(x1sq, x1g)
            sqp = ppr.tile([1, NPG], F32, tag="pair")
            mm2(sqp, ones64, x1sq)
            sq2s = wk.tile([1, NPG], F32, tag="sq2s")
            nc.scalar.copy(sq2s, sqp)
            u2p = ppr.tile([128, NPG], F32, tag="pair")
            mm2(u2p, w2d_s, x1g)
            u2s = wk.tile([128, NPG], F32, tag="u2s")
            nc.scalar.activation(u2s, u2p, IDENT, bias=b2_s)
            v2p = ppr.tile([128, NPG], F32, tag="pair")
            mm2(v2p, w2b_s, x1g)
            v2s = wk.tile([128, NPG], F32, tag="v2s")
            nc.scalar.copy(v2s, v2p)

            # ---- conv2 scores + topk ----
            asm2 = sb.tile([128, 128], U16, tag="asm")
            for t in range(8):
                sc = psc.tile([128, NPG], F32, tag="sc")
                lhs = x1g[:, 128 * t:128 * (t + 1)]
                for h in range(2):
                    o = sc[:, 512 * h:512 * (h + 1)]
                    nc.tensor.matmul(o, lhs, x1g[:, 512 * h:512 * (h + 1)],
                                     start=True, stop=False)
                    nc.tensor.matmul(o, neghalf,
                                     sq2s[:, 512 * h:512 * (h + 1)],
                                     start=False, stop=True)
                topk_tile(sc, asm2, t)
            idx_to_dram(asm2, idx2s, g)

            # ---- conv2 gather ----
            idxw2 = sb.tile([128, 640], I16, tag="idxw2")
            for u in range(8):
                srcap = bass.AP(idx2s, g * 10240, [[1, 16], [16, 640]])
                nc.sync.dma_start(out=idxw2[16 * u:16 * (u + 1), :], in_=srcap)
            vg2 = big.tile([128, K * NPG], F32, tag="gath")
            nc.gpsimd.ap_gather(vg2, v2s, idxw2, channels=128, num_elems=NPG,
                                d=1, num_idxs=K * NPG)

            # ---- conv2 pairs ----
            x2acc = pac.tile([128, NPG], F32, tag="acc")
            for k in range(K):
                zk = wk.tile([128, NPG], F32, tag="zk")
                nc.vector.tensor_add(zk, u2s, vg2[:, NPG * k:NPG * (k + 1)])
                hk = wk.tile([128, NPG], F32, tag="hk")
                nc.scalar.activation(hk, zk, LRELU, alpha=SLOPE)
                mm2(x2acc, identf, hk, start=(k == 0), stop=(k == K - 1))
            x2scr = wk.tile([128, NPG], F32, tag="zk")
            nc.scalar.activation(x2scr, x2acc, IDENT,
                                 accum_out=pooled2[:, g:g + 1])

        # ---------------- classifier (transposed, as baseline) ----------------
        def act(out_ap, in_ap, alpha, bias=0.0):
            if alpha == 1.0:
                nc.scalar.activation(out_ap, in_ap, IDENT, bias=bias)
            else:
                nc.scalar.activation(out_ap, in_ap, LRELU, bias=bias,
                                     alpha=alpha)

        p1 = cst.tile([128, 8 * GPC], F32, tag="p1")
        for m in range(8):
            pf = ppr.tile([128, GPC], F32, tag="pair")
            nc.tensor.matmul(pf, wlA_s[:, 128 * m:128 * (m + 1)], pooled1,
                             start=True, stop=False)
            nc.tensor.matmul(pf, wlB_s[:, 128 * m:128 * (m + 1)], pooled2,
                             start=False, stop=True)
            act(p1[:, GPC * m:GPC * (m + 1)], pf, 1.0, bias=blr_s[:, m:m + 1])
        p2 = cst.tile([128, 4 * GPC], F32, tag="p2")
        for m in range(4):
            pf2 = ppr.tile([128, GPC], F32, tag="pair")
            for kc in range(8):
                nc.tensor.matmul(
                    pf2, wm1_s[:, 512 * kc + 128 * m:512 * kc + 128 * (m + 1)],
                    p1[:, GPC * kc:GPC * (kc + 1)],
                    start=(kc == 0), stop=(kc == 7))
            act(p2[:, GPC * m:GPC * (m + 1)], pf2, SLOPE,
                bias=bm1_s[:, m:m + 1])
        p3 = cst.tile([128, 2 * GPC], F32, tag="p3")
        for m in range(2):
            pf3 = ppr.tile([128, GPC], F32, tag="pair")
            for kc in range(4):
                nc.tensor.matmul(
                    pf3, wm2_s[:, 256 * kc + 128 * m:256 * kc + 128 * (m + 1)],
                    p2[:, GPC * kc:GPC * (kc + 1)],
                    start=(kc == 0), stop=(kc == 3))
            act(p3[:, GPC * m:GPC * (m + 1)], pf3, SLOPE,
                bias=bm2_s[:, m:m + 1])
        pf4 = ppr.tile([3, GPC], F32, tag="pair")
        for kc in range(2):
            nc.tensor.matmul(pf4, wm3_s[:, 3 * kc:3 * (kc + 1)],
                             p3[:, GPC * kc:GPC * (kc + 1)],
                             start=(kc == 0), stop=(kc == 1))
        outs = cst.tile([3, GPC], F32, tag="outs")
        act(outs, pf4, 1.0, bias=bm3_s)
        nc.sync.dma_start(out=out, in_=outs)
        ctx.close()

    nc.compile()
    return nc


def prep_common(inputs):
    """Weight-derived tensors shared by all cores."""
    f32 = np.float32
    g = lambda k: np.asarray(inputs[k], f32)
    w1a, b1a = g("w1a"), g("b1a")
    w1b, b1b = g("w1b"), g("b1b")
    w1c, b1c = g("w1c"), g("b1c")
    w2, b2 = g("w2"), g("b2")
    wl, bl = g("wl"), g("bl")
    wm1, bm1 = g("wm1"), g("bm1")
    wm2, bm2 = g("wm2"), g("bm2")
    wm3, bm3 = g("wm3"), g("bm3")
    C = lambda a: np.ascontiguousarray(a, f32)
    return {
        "w1d": C(w1a[:4] - w1a[4:]),
        "w1bot": C(w1a[4:]),
        "w1bw": C(w1b), "w1cw": C(w1c),
        "b1a": C(b1a.reshape(64, 1)), "b1b": C(b1b.reshape(64, 1)),
        "b1c": C(b1c.reshape(64, 1)),
        "w2d": C(w2[:64] - w2[64:]), "w2b": C(w2[64:]),
        "b2": C(b2.reshape(128, 1)),
        "wlA": C(wl[:64] / NPG), "wlB": C(wl[64:] / NPG),
        "blr": C(bl.reshape(8, 128).T),
        "wm1r": C(wm1.reshape(8, 128, 512).transpose(1, 0, 2).reshape(128, 4096)),
        "bm1r": C(bm1.reshape(4, 128).T),
        "wm2r": C(wm2.reshape(4, 128, 256).transpose(1, 0, 2).reshape(128, 1024)),
        "bm2r": C(bm2.reshape(2, 128).T),
        "wm3r": C(wm3.reshape(2, 128, 3).transpose(1, 0, 2).reshape(128, 6)),
        "bm3r": C(bm3.reshape(3, 1)),
        "ident16": np.eye(128, dtype=np.float16),
    }


_CACHE = {}


class _Runtime:
    def __init__(self):
        import jax
        from jax.sharding import Mesh, PartitionSpec, NamedSharding
        from jax.experimental.shard_map import shard_map
        import concourse.mybir as mybir
        from concourse.bass2jax import (_bass_exec_p, install_neuronx_cc_hook,
                                        partition_id_tensor)

        self.jax = jax
        nc = build()
        self.nc = nc
        install_neuronx_cc_hook()
        partition_name = (nc.partition_id_tensor.name
                          if nc.partition_id_tensor else None)
        in_names, out_names, out_avals, zero_outs = [], [], [], []
        for alloc in nc.m.functions[0].allocations:
            if not isinstance(alloc, mybir.MemoryLocationSet):
                continue
            name = alloc.memorylocations[0].name
            if alloc.kind == "ExternalInput":
                if name != partition_name:
                    in_names.append(name)
            elif alloc.kind == "ExternalOutput":
                shape = tuple(alloc.tensor_shape)
                dtype = mybir.dt.np(alloc.dtype)
                out_names.append(name)
                out_avals.append(jax.core.ShapedArray(shape, dtype))
                zero_outs.append(np.zeros(shape, dtype))
        self.in_names = in_names
        self.out_shape = out_avals[0].shape
        n_params = len(in_names)
        n_outs = len(out_avals)
        all_in = in_names + out_names + ([partition_name] if partition_name
                                         else [])

        def _body(*args):
            operands = list(args)
            if partition_name is not None:
                operands.append(partition_id_tensor())
            return tuple(_bass_exec_p.bind(
                *operands, out_avals=tuple(out_avals), in_names=tuple(all_in),
                out_names=tuple(out_names), lowering_input_output_aliases=(),
                sim_require_finite=True, sim_require_nnan=True, nc=nc))

        devices = jax.devices()[:N_CORES]
        mesh = Mesh(np.asarray(devices), ("core",))
        self.sharding = NamedSharding(mesh, PartitionSpec("core"))
        self.sharded = jax.jit(
            shard_map(_body, mesh=mesh,
                      in_specs=(PartitionSpec("core"),) * (n_params + n_outs),
                      out_specs=(PartitionSpec("core"),) * n_outs,
                      check_rep=False),
            keep_unused=True)
        self.dev_zeros = [jax.device_put(
            np.zeros((N_CORES * z.shape[0], *z.shape[1:]), z.dtype),
            self.sharding) for z in zero_outs]
        self.whash = None
        self.dev_weights = None

    def _rep(self, a):
        """Replicate a per-core array 8x along axis 0 and device_put."""
        cat = np.ascontiguousarray(
            np.broadcast_to(a[None], (N_CORES,) + a.shape)
            .reshape(N_CORES * a.shape[0], *a.shape[1:]))
        return self.jax.device_put(cat, self.sharding)

    def run(self, inputs):
        # Build + launch the F1 transfer first (device_put is async), then
        # hash the weights while it is in flight. (A device-side F1 cache was
        # tried and measured SLOWER: reusing the buffer serializes against the
        # previous call, while a fresh transfer pipelines with dispatch.)
        f32 = np.float32
        xx = np.concatenate([np.asarray(inputs["x"], f32),
                             np.asarray(inputs["pos"], f32)], 1)
        n = GPC * NPG
        F1cat = np.empty((N_CORES * 5, n), f32)
        for c in range(N_CORES):
            sl = xx[c * n:(c + 1) * n]
            F1cat[c * 5:c * 5 + 4] = sl.T
            F1cat[c * 5 + 4] = (sl * sl).sum(1)
        dev_F1 = self.jax.device_put(F1cat, self.sharding)

        hh = hashlib.blake2b(digest_size=16)
        for k in ("w1a", "b1a", "w1b", "b1b", "w1c", "b1c", "w2", "b2",
                  "wl", "bl", "wm1", "bm1", "wm2", "bm2", "wm3", "bm3"):
            a = np.asarray(inputs[k])
            if not a.flags.c_contiguous:
                a = np.ascontiguousarray(a)
            hh.update(a)
        h = hh.digest()
        if self.whash != h:
            common = prep_common(inputs)
            self.dev_weights = {n: self._rep(common[n]) for n in common}
            self.whash = h
        args = [dev_F1 if nm == "F1" else self.dev_weights[nm]
                for nm in self.in_names]
        outs = self.sharded(*args, *self.dev_zeros)
        res = np.asarray(outs[0])  # [N_CORES*3, GPC]
        per = res.reshape(N_CORES, *self.out_shape)
        return np.concatenate([per[c].T for c in range(N_CORES)],
                              axis=0).astype(np.float32)


def kernel(x, pos, batch, w1a, b1a, w1b, b1b, w1c, b1c, w2, b2,
           wl, bl, wm1, bm1, wm2, bm2, wm3, bm3):
    if "rt" not in _CACHE:
        _CACHE["rt"] = _Runtime()
    return _CACHE["rt"].run(dict(
        x=x, pos=pos, batch=batch, w1a=w1a, b1a=b1a, w1b=w1b, b1b=b1b,
        w1c=w1c, b1c=b1c, w2=w2, b2=b2, wl=wl, bl=bl, wm1=wm1, bm1=bm1,
        wm2=wm2, bm2=bm2, wm3=wm3, bm3=bm3))

